# revision 1
# baseline (speedup 1.0000x reference)
"""Trainium2 Bass kernel for nn_C2f_DualModal_MoE (top-1 MoE over 1x1 convs).

Reference computation (per token t of N = B*H*W, channels C1 -> C2):
    logits = t @ Wr                  [N, E=4]
    idx    = argmax(softmax(logits)) = argmax(logits)   (top_k = 1)
    out    = t @ We[idx] + t @ Ws[0] = t @ (We[idx] + Ws[0])

Strategy (8 NeuronCores, data-parallel over batch, 1 image per core):
  - Everything stays channel-major: x[b] is [C1, H*W] in DRAM which is
    exactly the lhs-transposed / stream layout the PE wants. The output
    [C2, H*W] is produced directly in its DRAM layout. Zero transposes.
  - Shared expert folded into the routed weights: W'_e = We[e] + Ws[0].
  - Top-1 selection is folded into the GEMMs with a bilinear bit-mask
    decomposition. With idx = 2a + b (a,b in {0,1}):
        W'_idx = M0 + a*Ma + b*Mb + (ab)*Mab
        M0 = W'_0,  Ma = W'_2 - W'_0,  Mb = W'_1 - W'_0,
        Mab = W'_3 - W'_2 - W'_1 + W'_0
    so   out = x@M0 + (a.x)@Ma + (b.x)@Mb + (ab.x)@Mab
    where a/b are per-token {0,1} masks broadcast along channels
    (the ab stream is built by masking the b stream with a).
    All four matmuls accumulate into one PSUM tile -> no output combine.
  - Router runs in exact fp32 (token-major logits via x-stationary
    matmuls), expert GEMMs run in bf16 (inputs are exact {0,1}-masked
    bf16 casts; accumulation is fp32 in PSUM).
"""

import contextlib

import numpy as np

import concourse.bacc as bacc
import concourse.bass as bass
import concourse.mybir as mybir
import concourse.tile as tile
from concourse import bass_utils

P = 128
B = 8
C1 = 256
C2 = 256
E = 4
HW = 6400  # 80 * 80
KO = C1 // P  # k blocks (contraction)
MO = C2 // P  # m blocks (output channels)

NBLK = 10            # token blocks per image
BLK = HW // NBLK     # 640 tokens per block
RST = BLK // P       # 5 router sub-chunks per block (token stride trick)
CHUNK = 320          # GEMM token chunk (2 per block, 1 PSUM bank each)
NCH = BLK // CHUNK

F32 = mybir.dt.float32
BF16 = mybir.dt.bfloat16

STREAM_DT = BF16  # dtype of the expert GEMM streams / weights

import os

# debug bisect: full | norouter | norowdma | nobcast
DEBUG_STAGE = os.environ.get("MOE_DEBUG_STAGE", "full")


def _emit_once(nc, pools, aps, rep):
    pp, rpool, spool, opool, psum, rpsum, dpool, bcps = pools
    x, wr, we, ws, out = aps

    # ---------------- x (resident, fp32, block-major) ----------------
    # Block-major layout keeps every block's bytes contiguous so the tile
    # scheduler's access-range tracking sees the 10 loads as disjoint.
    x_sb = pp.tile([P, NBLK, KO, BLK], F32, tag="x_sb", name=f"x_sb_{rep}")
    xv = x.rearrange("(ko ki) f -> ki ko f", ki=P)

    # router weights + first x block first: they unblock the pipeline
    wr_sb = pp.tile([P, KO, E], F32, tag="wr_sb", name=f"wr_sb_{rep}")
    nc.sync.dma_start(wr_sb[:], wr.rearrange("(ko ki) e -> ki ko e", ki=P))
    for b in range(0, NBLK):
        nc.sync.dma_start(x_sb[:, b], xv[:, :, b * BLK : (b + 1) * BLK])

    # ---------------- weights (on the gpsimd queue, off SP) ----------
    we_sb = pp.tile([P, E, KO, C2], F32, tag="we_sb", name=f"we_sb_{rep}")
    ws_sb = pp.tile([P, KO, C2], F32, tag="ws_sb", name=f"ws_sb_{rep}")
    nc.gpsimd.dma_start(we_sb[:], we.rearrange("e (ko ki) d -> ki e ko d", ki=P))
    nc.gpsimd.dma_start(ws_sb[:], ws.rearrange("s (ko ki) d -> ki (s ko) d", ki=P))

    # fused W'_e = We[e] + Ws[0] -- one-time prep on the otherwise-idle
    # GPSIMD engine so DVE stays free during the x-load window
    wf = pp.tile([P, E, KO, C2], F32, tag="wf", name=f"wf_{rep}")
    for e in range(E):
        nc.gpsimd.tensor_tensor(wf[:, e], we_sb[:, e], ws_sb[:], mybir.AluOpType.add)

    # bilinear matrices, in stream dtype, stream index s: 0->M0 (x),
    # 1->Ma (a.x), 2->Mb (b.x), 3->Mab (ab.x)
    msb = pp.tile([P, 4, KO, C2], STREAM_DT, tag="msb", name=f"msb_{rep}")
    mb_f = pp.tile([P, KO, C2], F32, tag="mb_f", name=f"mb_f_{rep}")
    t1_f = pp.tile([P, KO, C2], F32, tag="t1_f", name=f"t1_f_{rep}")
    nc.gpsimd.tensor_copy(msb[:, 0], wf[:, 0])
    nc.gpsimd.tensor_tensor(t1_f[:], wf[:, 2], wf[:, 0], mybir.AluOpType.subtract)
    nc.gpsimd.tensor_copy(msb[:, 1], t1_f[:])
    nc.gpsimd.tensor_tensor(mb_f[:], wf[:, 1], wf[:, 0], mybir.AluOpType.subtract)
    nc.gpsimd.tensor_copy(msb[:, 2], mb_f[:])
    nc.gpsimd.tensor_tensor(t1_f[:], wf[:, 3], wf[:, 2], mybir.AluOpType.subtract)
    nc.gpsimd.tensor_tensor(t1_f[:], t1_f[:], mb_f[:], mybir.AluOpType.subtract)
    nc.gpsimd.tensor_copy(msb[:, 3], t1_f[:])

    # descending weights [4,3,2,1] used to pick the FIRST argmax on ties
    w4 = pp.tile([P, E], F32, tag="w4", name=f"w4_{rep}")
    for j in range(E):
        nc.vector.memset(w4[:, j : j + 1], float(E - j))

    # bit-mask rows [2, HW] (partition 0 = a, partition 1 = b) and the
    # channel-broadcast masks [P, 2, HW] (only a and b; ab = a*(b*x))
    row2_dram = dpool.tile([2, HW], STREAM_DT, tag="row2", name=f"row2_{rep}")
    bc2 = pp.tile([P, 2, HW], STREAM_DT, tag="bc2", name=f"bc2_{rep}")

    out_v = out.rearrange("(mo mi) f -> mi mo f", mi=P)

    if DEBUG_STAGE == "norouter":
        nc.vector.memset(bc2[:], 1.0)
    # ---------------- per-block router ----------------
    # Block b covers tokens [b*BLK, (b+1)*BLK). Within the block, router
    # sub-chunk i handles tokens {b*BLK + RST*p + i : p in [0,128)} so the
    # token-major [128, RST] index tile flattens contiguously to a row.
    for b in range(NBLK) if DEBUG_STAGE != "norouter" else []:
        bsl = slice(b * BLK, (b + 1) * BLK)
        # logits, token-major: [128 tokens, RST, E]
        pr = rpsum.tile([P, RST, E], F32, tag="pr", name=f"pr_{rep}_{b}")
        xb_r = x_sb[:, b].rearrange("ki ko (p r) -> ki ko r p", r=RST)
        for i in range(RST):
            for k in range(KO):
                nc.tensor.matmul(
                    pr[:, i, :],
                    xb_r[:, k, i, :],
                    wr_sb[:, k, :],
                    start=(k == 0),
                    stop=(k == KO - 1),
                )
        lg = rpool.tile([P, RST, E], F32, tag="lg", name=f"lg_{rep}_{b}")
        nc.vector.tensor_copy(lg[:], pr[:])
        mx = rpool.tile([P, RST], F32, tag="mx", name=f"mx_{rep}_{b}")
        nc.vector.reduce_max(mx[:], lg[:], axis=mybir.AxisListType.X)
        eq = rpool.tile([P, RST, E], F32, tag="eq", name=f"eq_{rep}_{b}")
        nc.vector.tensor_tensor(
            eq[:],
            lg[:],
            mx[:, :, None].to_broadcast((P, RST, E)),
            mybir.AluOpType.is_equal,
        )
        nc.vector.tensor_tensor(
            eq[:],
            eq[:],
            w4[:, None, :].to_broadcast((P, RST, E)),
            mybir.AluOpType.mult,
        )
        rmax = rpool.tile([P, RST], F32, tag="rmax", name=f"rmax_{rep}_{b}")
        nc.vector.reduce_max(rmax[:], eq[:], axis=mybir.AxisListType.X)
        # idx = E - rmax in {0..3}; a = bit1, b = bit0
        idx = rpool.tile([P, RST], F32, tag="idx", name=f"idx_{rep}_{b}")
        nc.vector.tensor_scalar(
            idx[:], rmax[:], -1.0, float(E), mybir.AluOpType.mult, mybir.AluOpType.add
        )
        a_f = rpool.tile([P, RST], F32, tag="a_f", name=f"a_f_{rep}_{b}")
        nc.vector.tensor_scalar(a_f[:], idx[:], 2.0, None, mybir.AluOpType.is_ge)
        b_f = rpool.tile([P, RST], F32, tag="b_f", name=f"b_f_{rep}_{b}")
        nc.vector.tensor_scalar(b_f[:], a_f[:], -2.0, None, mybir.AluOpType.mult)
        nc.vector.tensor_tensor(b_f[:], b_f[:], idx[:], mybir.AluOpType.add)
        amh = rpool.tile([P, 2, RST], STREAM_DT, tag="amh", name=f"amh_{rep}_{b}")
        nc.vector.tensor_copy(amh[:, 0], a_f[:])
        nc.vector.tensor_copy(amh[:, 1], b_f[:])

        if DEBUG_STAGE == "norowdma":
            if b == 0:
                nc.vector.memset(bc2[:], 1.0)
            continue
        # token-major [128, 2, RST] -> DRAM rows [2, BLK] (token-contig)
        nc.gpsimd.dma_start(
            row2_dram[:, bsl].rearrange("t (p r) -> p t r", r=RST), amh[:]
        )
        if DEBUG_STAGE == "nobcast":
            if b == 0:
                nc.vector.memset(bc2[:], 1.0)
            continue
        # broadcast-read DRAM rows across 128 partitions (SWDGE: the
        # HWDGE path hangs on zero-step source dims)
        nc.gpsimd.dma_start(
            bc2[:, :, bsl], row2_dram[None, :, bsl].to_broadcast((P, 2, BLK))
        )

    # ---------------- expert GEMMs ----------------
    for b in range(NBLK):
        bsl = slice(b * BLK, (b + 1) * BLK)
        stgs = [
            opool.tile([P, BLK], F32, tag=f"stg{m}", name=f"stg{m}_{rep}_{b}")
            for m in range(MO)
        ]
        for c in range(NCH):
            csl = slice(b * BLK + c * CHUNK, b * BLK + (c + 1) * CHUNK)
            lsl = slice(c * CHUNK, (c + 1) * CHUNK)
            xs = spool.tile([P, KO, CHUNK], STREAM_DT, tag="xs", name=f"xs_{rep}_{b}_{c}")
            nc.vector.tensor_copy(xs[:], x_sb[:, b, :, lsl])
            xa = spool.tile([P, KO, CHUNK], STREAM_DT, tag="xa", name=f"xa_{rep}_{b}_{c}")
            nc.vector.tensor_tensor(
                xa[:],
                xs[:],
                bc2[:, 0, None, csl].to_broadcast((P, KO, CHUNK)),
                mybir.AluOpType.mult,
            )
            xb = spool.tile([P, KO, CHUNK], STREAM_DT, tag="xb", name=f"xb_{rep}_{b}_{c}")
            nc.vector.tensor_tensor(
                xb[:],
                xs[:],
                bc2[:, 1, None, csl].to_broadcast((P, KO, CHUNK)),
                mybir.AluOpType.mult,
            )
            xab = spool.tile(
                [P, KO, CHUNK], STREAM_DT, tag="xab", name=f"xab_{rep}_{b}_{c}"
            )
            nc.vector.tensor_tensor(
                xab[:],
                xb[:],
                bc2[:, 0, None, csl].to_broadcast((P, KO, CHUNK)),
                mybir.AluOpType.mult,
            )
            streams = [xs, xa, xb, xab]
            for m in range(MO):
                po = psum.tile([P, CHUNK], F32, tag="po", name=f"po_{rep}_{b}_{c}_{m}")
                for s in range(4):
                    for k in range(KO):
                        nc.tensor.matmul(
                            po[:],
                            msb[:, s, k, m * P : (m + 1) * P],
                            streams[s][:, k, :],
                            start=(s == 0 and k == 0),
                            stop=(s == 3 and k == KO - 1),
                        )
                nc.scalar.activation(
                    stgs[m][:, c * CHUNK : (c + 1) * CHUNK],
                    po[:],
                    mybir.ActivationFunctionType.Copy,
                )
        for m in range(MO):
            nc.sync.dma_start(out_v[:, m, bsl], stgs[m][:])


def _build_body(tc, x, wr, we, ws, out, loop_n=None):
    nc = tc.nc
    ctx = contextlib.ExitStack()
    with ctx:
        pp = ctx.enter_context(tc.tile_pool(name="persist", bufs=1))
        rpool = ctx.enter_context(tc.tile_pool(name="router", bufs=2))
        spool = ctx.enter_context(tc.tile_pool(name="streams", bufs=4))
        opool = ctx.enter_context(tc.tile_pool(name="outstg", bufs=3))
        psum = ctx.enter_context(tc.tile_pool(name="psum", bufs=4, space="PSUM"))
        rpsum = ctx.enter_context(tc.tile_pool(name="rpsum", bufs=2, space="PSUM"))
        dpool = ctx.enter_context(tc.tile_pool(name="dram", bufs=1, space="DRAM"))
        bcps = ctx.enter_context(tc.tile_pool(name="bcpsum", bufs=2, space="PSUM"))
        pools = (pp, rpool, spool, opool, psum, rpsum, dpool, bcps)
        if isinstance(loop_n, str) and loop_n.startswith("for"):
            n_iter = int(loop_n[3:])
            with tc.For_i(0, n_iter, 1):
                _emit_once(nc, pools, (x, wr, we, ws, out), 0)
        else:
            for rep in range(loop_n or 1):
                _emit_once(nc, pools, (x, wr, we, ws, out), rep)


_NC_CACHE = {}


def _get_nc(loop_n=None):
    key = ("nc", loop_n)
    if key not in _NC_CACHE:
        nc = bacc.Bacc("TRN2", debug=False, num_swdge_queues=4)
        x = nc.dram_tensor("x", [C1, HW], F32, kind="ExternalInput").ap()
        wr = nc.dram_tensor("wr", [C1, E], F32, kind="ExternalInput").ap()
        we = nc.dram_tensor("we", [E, C1, C2], F32, kind="ExternalInput").ap()
        ws = nc.dram_tensor("ws", [1, C1, C2], F32, kind="ExternalInput").ap()
        out = nc.dram_tensor("out", [C2, HW], F32, kind="ExternalOutput").ap()
        with tile.TileContext(nc) as tc:
            _build_body(tc, x, wr, we, ws, out, loop_n=loop_n)
        nc.compile()
        _NC_CACHE[key] = nc
    return _NC_CACHE[key]


def kernel(x, Wr, We, Ws, top_k, _trace=False):
    assert int(top_k) == 1, "kernel hardcodes top_k == 1"
    x = np.ascontiguousarray(np.asarray(x, dtype=np.float32))
    Wr_n = np.ascontiguousarray(np.asarray(Wr, dtype=np.float32))
    We_n = np.ascontiguousarray(np.asarray(We, dtype=np.float32))
    Ws_n = np.ascontiguousarray(np.asarray(Ws, dtype=np.float32))
    b, c1, h, w = x.shape
    assert (b, c1, h * w) == (B, C1, HW)

    nc = _get_nc()
    in_maps = [
        {
            "x": x[core].reshape(C1, HW),
            "wr": Wr_n,
            "we": We_n,
            "ws": Ws_n,
        }
        for core in range(B)
    ]
    res = bass_utils.run_bass_kernel_spmd(
        nc, in_maps, core_ids=list(range(B)), trace=_trace
    )
    outs = np.stack([res.results[core]["out"] for core in range(B)])
    out = outs.reshape(B, C2, h, w)
    if _trace:
        return out, res
    return out



# revision 7
# speedup vs baseline: 1.0068x; 1.0068x over previous
"""Trainium2 Bass kernel for nn_C2f_DualModal_MoE (top-1 MoE over 1x1 convs).

Reference computation (per token t of N = B*H*W, channels C1 -> C2):
    logits = t @ Wr                  [N, E=4]
    idx    = argmax(softmax(logits)) = argmax(logits)   (top_k = 1)
    out    = t @ We[idx] + t @ Ws[0] = t @ (We[idx] + Ws[0])

Strategy (8 NeuronCores, data-parallel over batch, 1 image per core):
  - Channel-major end to end: x[b] is [C1, H*W] in DRAM = the exact
    stream layout the PE wants; out [C2, H*W] written directly.
  - Host-side weight fusion: W'_e = We[e] + Ws[0], then the bilinear
    bit-mask basis (idx = 2a + b):
        M0 = W'_0, Ma = W'_2 - W'_0, Mb = W'_1 - W'_0,
        Mab = W'_3 - W'_2 - W'_1 + W'_0
    uploaded pre-cast to bf16 as msb [C1, 4, C2]. No on-device prep.
  - out = x@M0 + (a.x)@Ma + (b.x)@Mb + (ab.x)@Mab, all 8 (k,s) matmuls
    accumulate into one PSUM tile per (chunk, m).
  - Router exact fp32 (token-major via x-stationary matmuls); masks are
    derived with a 6-op DVE chain reading router PSUM directly:
        m_j  = max(l_{2j}, l_{2j+1})          (paired TT, j=0,1)
        c_j  = (l_{2j+1} > l_{2j})
        a    = (m_1 > m_0)
        b    = c_0 + a*(c_1 - c_0)
    Ties resolve to the lower index, matching jax.lax.top_k.
  - Masks go token-major [128, 2, RST] -> row layout [2, GT] (strided
    HWDGE scatter on the act queue) -> partition-broadcast [128, 2, GT]
    (SWDGE handles the zero-step source). Streams are all-bf16 DVE
    multiplies (2x mode); the fp32->bf16 cast of x runs at 2x_2p.
  - 5 pipeline groups of 1280 tokens; PE order r0,r1,g0,r2,g1,... keeps
    the array busy during the mask/broadcast latency of the next group.
"""

import contextlib
import os

import numpy as np
import ml_dtypes

import concourse.bacc as bacc
import concourse.bass as bass
import concourse.mybir as mybir
import concourse.tile as tile
from concourse import bass_utils

P = 128
B = 8
C1 = 256
C2 = 256
E = 4
HW = 6400  # 80 * 80
KO = C1 // P  # contraction blocks
MO = C2 // P  # output-channel blocks

NG = 5               # pipeline groups per image
GT = HW // NG        # 1280 tokens per group
RST = GT // P        # 10 tokens per partition per group
CHUNK = 320          # GEMM token chunk (1 PSUM bank each)
NCH = GT // CHUNK    # 4 chunks per group

F32 = mybir.dt.float32
BF16 = mybir.dt.bfloat16

# debug bisect: full | norouter (masks forced to 1, skips router+bcast)
DEBUG_STAGE = os.environ.get("MOE_DEBUG_STAGE", "full")


def _emit_once(nc, pools, aps, rep):
    pp, xpool, mpool, bcpool, spool, opool, psum, rpsum, dpool = pools
    x, wr, msb, out = aps

    # ---------------- resident loads ----------------
    wr_sb = pp.tile([P, KO, E], F32, tag="wr_sb", name=f"wr_sb_{rep}")
    nc.sync.dma_start(wr_sb[:], wr.rearrange("(ko ki) e -> ki ko e", ki=P))

    # fused bilinear weights, bf16, loaded on the gpsimd queue (idle early)
    msb_sb = pp.tile([P, 4, KO, C2], BF16, tag="msb_sb", name=f"msb_sb_{rep}")
    nc.gpsimd.dma_start(msb_sb[:], msb.rearrange("(ko ki) s d -> ki s ko d", ki=P))

    xv = x.rearrange("(ko ki) f -> ki ko f", ki=P)
    out_v = out.rearrange("(mo mi) f -> mi mo f", mi=P)

    x_g = [None] * NG
    for g in range(NG):
        x_g[g] = xpool.tile([P, KO, GT], F32, tag=f"x_g{g % 3}", name=f"x_{rep}_{g}")
        nc.sync.dma_start(x_g[g][:], xv[:, :, g * GT : (g + 1) * GT])

    # one DRAM row buffer for the whole image; per-group slices are disjoint
    rows_hw = dpool.tile([2, HW], BF16, tag="rows", name=f"rows_{rep}")
    bc2 = [None] * NG

    def emit_router(g):
        """Router logits + masks + broadcast for group g."""
        if DEBUG_STAGE == "norouter":
            bc2[g] = bcpool.tile([P, 2, GT], BF16, tag="bc2", name=f"bc2_{rep}_{g}")
            nc.vector.memset(bc2[g][:], 1.0)
            return
        # token-major logits: partition p, chunk i -> token g*GT + p*RST + i
        pr = rpsum.tile([P, RST, E], F32, tag="pr", name=f"pr_{rep}_{g}")
        xr = x_g[g].rearrange("ki ko (p r) -> ki ko r p", r=RST)
        for i in range(RST):
            for k in range(KO):
                nc.tensor.matmul(
                    pr[:, i, :],
                    xr[:, k, i, :],
                    wr_sb[:, k, :],
                    start=(k == 0),
                    stop=(k == KO - 1),
                )
        # PSUM -> SBUF once (TT ops cannot read two PSUM operands)
        lg = mpool.tile([P, RST, E], F32, tag="lg", name=f"lg_{rep}_{g}")
        nc.vector.tensor_copy(lg[:], pr[:])
        lgv = lg.rearrange("p r (two e) -> p r two e", e=2)
        # paired ops: j=0 -> (l0,l1), j=1 -> (l2,l3)
        mx = mpool.tile([P, RST, 2], F32, tag="mx", name=f"mx_{rep}_{g}")
        nc.vector.tensor_tensor(
            mx[:], lgv[:, :, :, 0], lgv[:, :, :, 1], mybir.AluOpType.max
        )
        cmp = mpool.tile([P, RST, 2], F32, tag="cmp", name=f"cmp_{rep}_{g}")
        nc.vector.tensor_tensor(
            cmp[:], lgv[:, :, :, 1], lgv[:, :, :, 0], mybir.AluOpType.is_gt
        )
        amh = mpool.tile([P, 2, RST], BF16, tag="amh", name=f"amh_{rep}_{g}")
        a_f = mpool.tile([P, RST], F32, tag="a_f", name=f"a_f_{rep}_{g}")
        nc.vector.tensor_tensor(
            a_f[:], mx[:, :, 1], mx[:, :, 0], mybir.AluOpType.is_gt
        )
        nc.vector.tensor_copy(amh[:, 0], a_f[:])
        d_f = mpool.tile([P, RST], F32, tag="d_f", name=f"d_f_{rep}_{g}")
        nc.vector.tensor_tensor(
            d_f[:], cmp[:, :, 1], cmp[:, :, 0], mybir.AluOpType.subtract
        )
        nc.vector.tensor_tensor(d_f[:], d_f[:], a_f[:], mybir.AluOpType.mult)
        nc.vector.tensor_tensor(
            amh[:, 1], d_f[:], cmp[:, :, 0], mybir.AluOpType.add
        )
        # token-major [128, 2, RST] -> DRAM rows [2, GT] (token-contiguous
        # runs of RST per partition); strided scatter on the act HWDGE
        # queue, then a partition-broadcast read back on SWDGE (the only
        # DGE that handles the zero-step source dims).
        bc2[g] = bcpool.tile([P, 2, GT], BF16, tag="bc2", name=f"bc2_{rep}_{g}")
        gsl = slice(g * GT, (g + 1) * GT)
        nc.scalar.dma_start(
            rows_hw[:, gsl].rearrange("t (p r) -> p t r", r=RST), amh[:]
        )
        nc.gpsimd.dma_start(
            bc2[g][:], rows_hw[None, :, gsl].to_broadcast((P, 2, GT))
        )

    def emit_gemm(g):
        """Streams + expert GEMMs + output for group g."""
        gsl = slice(g * GT, (g + 1) * GT)
        xs = spool.tile([P, KO, GT], BF16, tag="xs", name=f"xs_{rep}_{g}")
        nc.vector.tensor_copy(xs[:], x_g[g][:])
        bca = bc2[g][:, 0, None, :].to_broadcast((P, KO, GT))
        bcb = bc2[g][:, 1, None, :].to_broadcast((P, KO, GT))
        xa = spool.tile([P, KO, GT], BF16, tag="xa", name=f"xa_{rep}_{g}")
        nc.vector.tensor_tensor(xa[:], xs[:], bca, mybir.AluOpType.mult)
        xb = spool.tile([P, KO, GT], BF16, tag="xb", name=f"xb_{rep}_{g}")
        nc.vector.tensor_tensor(xb[:], xs[:], bcb, mybir.AluOpType.mult)
        xab = spool.tile([P, KO, GT], BF16, tag="xab", name=f"xab_{rep}_{g}")
        nc.vector.tensor_tensor(xab[:], xb[:], bca, mybir.AluOpType.mult)
        streams = [xs, xa, xb, xab]

        out_sb = opool.tile([P, MO, GT], F32, tag="out_sb", name=f"osb_{rep}_{g}")
        for c in range(NCH):
            csl = slice(c * CHUNK, (c + 1) * CHUNK)
            for m in range(MO):
                po = psum.tile([P, CHUNK], F32, tag="po", name=f"po_{rep}_{g}_{c}_{m}")
                for s in range(4):
                    for k in range(KO):
                        nc.tensor.matmul(
                            po[:],
                            msb_sb[:, s, k, m * P : (m + 1) * P],
                            streams[s][:, k, csl],
                            start=(s == 0 and k == 0),
                            stop=(s == 3 and k == KO - 1),
                        )
                nc.scalar.activation(
                    out_sb[:, m, csl], po[:], mybir.ActivationFunctionType.Copy
                )
        nc.scalar.dma_start(out_v[:, :, gsl], out_sb[:])

    # PE order: r0, r1, g0, r2, g1, r3, g2, r4, g3, g4 — the router of
    # group g+1 fills the mask/broadcast latency window of group g.
    emit_router(0)
    if NG > 1:
        emit_router(1)
    for g in range(NG):
        emit_gemm(g)
        if g + 2 < NG:
            emit_router(g + 2)


def _build_body(tc, x, wr, msb, out, loop_n=None):
    nc = tc.nc
    ctx = contextlib.ExitStack()
    with ctx:
        pp = ctx.enter_context(tc.tile_pool(name="persist", bufs=1))
        xpool = ctx.enter_context(tc.tile_pool(name="xin", bufs=2))
        mpool = ctx.enter_context(tc.tile_pool(name="masks", bufs=2))
        bcpool = ctx.enter_context(tc.tile_pool(name="bcast", bufs=2))
        spool = ctx.enter_context(tc.tile_pool(name="streams", bufs=2))
        opool = ctx.enter_context(tc.tile_pool(name="outstg", bufs=2))
        psum = ctx.enter_context(tc.tile_pool(name="psum", bufs=4, space="PSUM"))
        rpsum = ctx.enter_context(tc.tile_pool(name="rpsum", bufs=2, space="PSUM"))
        dpool = ctx.enter_context(tc.tile_pool(name="dram", bufs=1, space="DRAM"))
        pools = (pp, xpool, mpool, bcpool, spool, opool, psum, rpsum, dpool)
        if isinstance(loop_n, str) and loop_n.startswith("for"):
            n_iter = int(loop_n[3:])
            with tc.For_i(0, n_iter, 1):
                _emit_once(nc, pools, (x, wr, msb, out), 0)
        else:
            for rep in range(loop_n or 1):
                _emit_once(nc, pools, (x, wr, msb, out), rep)


_NC_CACHE = {}


def _get_nc(loop_n=None):
    key = ("nc", loop_n)
    if key not in _NC_CACHE:
        nc = bacc.Bacc("TRN2", debug=False, num_swdge_queues=4)
        x = nc.dram_tensor("x", [C1, HW], F32, kind="ExternalInput").ap()
        wr = nc.dram_tensor("wr", [C1, E], F32, kind="ExternalInput").ap()
        msb = nc.dram_tensor("msb", [C1, 4, C2], BF16, kind="ExternalInput").ap()
        out = nc.dram_tensor("out", [C2, HW], F32, kind="ExternalOutput").ap()
        with tile.TileContext(nc) as tc:
            _build_body(tc, x, wr, msb, out, loop_n=loop_n)
        nc.compile()
        _NC_CACHE[key] = nc
    return _NC_CACHE[key]


def _fuse_weights(Wr, We, Ws):
    """Host-side: shared-expert fold + bilinear basis, cast to bf16."""
    Wp = We.astype(np.float32) + Ws[0].astype(np.float32)[None]
    M0 = Wp[0]
    Ma = Wp[2] - Wp[0]
    Mb = Wp[1] - Wp[0]
    Mab = Wp[3] - Wp[2] - Wp[1] + Wp[0]
    M = np.stack([M0, Ma, Mb, Mab])  # [4, C1, C2], stream order
    return np.ascontiguousarray(M.transpose(1, 0, 2)).astype(ml_dtypes.bfloat16)


def kernel(x, Wr, We, Ws, top_k, _trace=False):
    assert int(top_k) == 1, "kernel hardcodes top_k == 1"
    x = np.ascontiguousarray(np.asarray(x, dtype=np.float32))
    Wr_n = np.ascontiguousarray(np.asarray(Wr, dtype=np.float32))
    We_n = np.ascontiguousarray(np.asarray(We, dtype=np.float32))
    Ws_n = np.ascontiguousarray(np.asarray(Ws, dtype=np.float32))
    b, c1, h, w = x.shape
    assert (b, c1, h * w) == (B, C1, HW)

    msb_np = _fuse_weights(Wr_n, We_n, Ws_n)

    nc = _get_nc()
    in_maps = [
        {
            "x": x[core].reshape(C1, HW),
            "wr": Wr_n,
            "msb": msb_np,
        }
        for core in range(B)
    ]
    res = bass_utils.run_bass_kernel_spmd(
        nc, in_maps, core_ids=list(range(B)), trace=_trace
    )
    outs = np.stack([res.results[core]["out"] for core in range(B)])
    out = outs.reshape(B, C2, h, w)
    if _trace:
        return out, res
    return out
